# revision 26
# baseline (speedup 1.0000x reference)
"""
Causal self-attention (single head) on 8 trn2 NeuronCores.

Problem: x[4, 2048, 1024], Wq/Wk/Wv[1024, 1024] (torch Linear layout
[d_out, d_in]).
    q/k/v = x @ W.T ; out = softmax(mask(q k^T) / 32) @ v

Key algebraic restructuring — fold Wq into Wk on the HOST:
    scores = (x Wq^T)(x Wk^T)^T = x (Wq^T Wk) x^T = x M x^T
M = Wq^T Wk is computed host-side in fp64 (1 GFLOP, free).  On-chip,
ONE projection  z = M x^T  over the core's own keys replaces BOTH the
q-projection (which was duplicated across the pair) and the
k-projection; the query side of the scores matmul reads the resident
xT tile directly.  This cuts the matmul work per core by ~1/3.

Sharding — flash-style key split (no collectives, uniform SPMD
program; all role differences live in the INPUTS):
  core c -> batch b = c // 2, role r = c % 2.
  Keys/values split by alternating 128-row j-tiles: core r owns global
  j-tiles {2t + r} and computes z/V only for its own 1024 key rows.
  Each core produces partial ctx = sum_j exp(s_j) v_j and partial
  den = sum_j exp(s_j) over ITS keys for ALL queries; the host
  combines:  out = (ctxE + ctxO) / (denE + denO).
  exp needs no running-max (logits/32 are O(2)).

  Host-side column permutation: x columns (sequence) are reordered
  OWN-tiles-first [own 8 x 128 | other 8 x 128].  z/V projections read
  xT[:, 0:1024] on every core (uniform program); query block ib is
  permuted row-tiles {ib, ib+8} on both roles, selected with one
  strided access pattern.  ctx/den come out in permuted row order; the
  host un-permutes.  The diagonal-tile causal mask is constant per
  role:  r=0: [tril | keep-all]   r=1: [tril | drop-all]  (bf16 0/1).

Precision: the z projection runs in fp8 e4m3 with the DoubleRow perf
mode (2 contraction tiles per instruction, 2x bf16 throughput; M is
pre-scaled by 32 to clear the e4m3 subnormal cliff, the exp scale
absorbs it).  Everything else is bf16 (~0.43 ns/row at N=512,
LDWEIGHTS hidden); PSUM accumulates fp32.  Simulated end-to-end
rel err: 1.56e-2 (Z_FP8) / 4.1e-3 (bf16 z) vs the 2e-2 gate; HW has
matched the numpy sim exactly on every previous config, and test.py
re-verifies on HW.  fp8 on scores or on the v path measured over the
gate — not used.

Schedule notes (from perfetto traces):
 - Engines/DMA only start ~8-10 us into the NEFF; the warm-up tile is
   memset on-chip and ~72 warm matmuls lift the PE clock gate through
   that window (shorter warm-ups let the clock drop and the whole
   kernel settles ~20% slow).
 - Measured per-queue DMA rates are wildly uneven: gpsimd (software
   DGE) ~200 GB/s, scalar ~80, sync ~44.  Critical loads (m8 + own-x8)
   ride gpsimd first; the rest are ordered by need date.
 - Attention is one flattened stream of (block, tile) steps,
   software-pipelined: AV+den of step k are issued AFTER the scores of
   step k+1 (across block boundaries), so every exp/mask chain and
   PSUM-bank WAR wait has a full scores window (~0.9 us) of cover
   (cross-engine semaphore observation latency is ~0.85 us).
 - i-blocks run largest-first so the final block's output DMA is tiny;
   in-block tile order is [0, diag, 1, ...]; ctx copies split
   vector/scalar; den partials accumulate in one resident SBUF tile,
   DMA'd once at the end.
"""

import sys

for _p in ("/opt/trn_rl_repo", "/root/.axon_site/_ro/trn_rl_repo"):
    if _p not in sys.path:
        sys.path.append(_p)

import numpy as np
import ml_dtypes

import concourse.bass as bass
import concourse.mybir as mybir
import concourse.tile as tile
from concourse import bacc
from concourse.bass_utils import run_bass_kernel_spmd

F32 = mybir.dt.float32
BF16 = mybir.dt.bfloat16
FP8 = mybir.dt.float8e4
NPBF16 = ml_dtypes.bfloat16
NPFP8 = ml_dtypes.float8_e4m3
DR = mybir.MatmulPerfMode.DoubleRow

Z_FP8 = True         # z = Mx^T in fp8-DoubleRow (1.56e-2) vs bf16 (4.1e-3)

B, S, D = 4, 2048, 1024
P = 128
ND = D // P          # 8 d-tiles (projection contraction)
NO = D // P          # 8 o-tiles
IB = 256             # query block rows
N_IB = S // IB       # 8 query blocks
JH = S // 2          # 1024 own key rows per core
NJT = JH // P        # 8 own j-tiles
N_CORES = 8
M_SCALE = 32.0 if Z_FP8 else 1.0


def _mm(nc, out, lhsT, rhs, start, stop):
    nc.tensor.matmul(out, lhsT, rhs, start=start, stop=stop)


def build_program():
    nc = bacc.Bacc(
        "TRN2",
        target_bir_lowering=False,
        debug=False,
        enable_asserts=False,
        num_devices=N_CORES,
    )
    # xT: full permuted sequence bf16 (scores query side + V projection);
    # x8/m8: z-projection operands (own key half only)
    xT = nc.dram_tensor("xT", [D, S], BF16, kind="ExternalInput").ap()
    zdt = FP8 if Z_FP8 else BF16
    x8 = nc.dram_tensor("x8", [D, JH], zdt, kind="ExternalInput").ap()
    m8 = nc.dram_tensor("m8", [D, D], zdt, kind="ExternalInput").ap()
    wvT = nc.dram_tensor("wvT", [D, D], BF16, kind="ExternalInput").ap()
    mask_in = nc.dram_tensor("mask", [P, IB], BF16, kind="ExternalInput").ap()
    ones_in = nc.dram_tensor("ones", [P, 2], BF16, kind="ExternalInput").ap()
    ctx_out = nc.dram_tensor("ctx", [S, D], BF16, kind="ExternalOutput").ap()
    den_out = nc.dram_tensor("den", [2, 16 * IB], F32, kind="ExternalOutput").ap()

    scale = 1.0 / (32.0 * M_SCALE)  # 1/sqrt(d_v), absorbing the M pre-scale

    def d_major(ap2d):
        # [ND*P, C] DRAM view -> [P, ND, C] (partition-major 3D AP)
        return ap2d.rearrange("(nd p) c -> p nd c", p=P)

    with tile.TileContext(nc) as tc:
        with tc.tile_pool(name="res", bufs=1) as rpool:
            # On-chip warm-up source: no DMA dependency, so the PE clock
            # gate lifts during the DMA spin-up dead window.
            warm = rpool.tile([P, 512], BF16, tag="warm", name="warm")
            nc.vector.memset(warm[:], 0.0)
            mask_t = rpool.tile([P, IB], BF16, tag="mask")
            nc.sync.dma_start(mask_t[:], mask_in[:])
            ones_t = rpool.tile([P, 2], BF16, tag="ones")
            nc.sync.dma_start(ones_t[:], ones_in[:])

            zt = rpool.tile([P, NO, JH], BF16, tag="zt", name="zt")
            v_t = rpool.tile([P, NJT, D], BF16, tag="v", name="v")
            den_all = rpool.tile([2, 16 * IB], F32, tag="den", name="den_all")
            xT_t = rpool.tile([P, ND, S], BF16, tag="xT", name="xT")

            # ---------------- Phase A: projections ----------------
            with (
                tc.tile_pool(name="xp", bufs=1) as xpool,
                tc.tile_pool(name="psA", bufs=4, space="PSUM") as psA,
            ):
                # Queue plan by need date (gpsimd is the fast queue).
                if Z_FP8:
                    x8_t = xpool.tile([P, ND // 2, 2, JH], FP8, tag="x8",
                                      name="x8")
                    x8_v = d_major(x8).rearrange(
                        "p (d2 two) c -> p d2 two c", two=2)
                    m8_t = xpool.tile([P, ND // 2, 2, D], FP8, tag="m8",
                                      name="m8")
                    m8_v = d_major(m8).rearrange(
                        "p (d2 two) c -> p d2 two c", two=2)
                    nc.gpsimd.dma_start(m8_t[:, 0:2], m8_v[:, 0:2])
                    nc.scalar.dma_start(m8_t[:, 2:3], m8_v[:, 2:3])
                    nc.sync.dma_start(m8_t[:, 3:4], m8_v[:, 3:4])
                    nc.gpsimd.dma_start(x8_t[:, :, :, 0:512],
                                        x8_v[:, :, :, 0:512])
                    nc.gpsimd.dma_start(x8_t[:, :, :, 512:JH],
                                        x8_v[:, :, :, 512:JH])
                else:
                    x8_t = xpool.tile([P, ND, JH], BF16, tag="x8", name="x8")
                    m8_t = xpool.tile([P, ND, D], BF16, tag="m8", name="m8")
                    nc.gpsimd.dma_start(m8_t[:], d_major(m8))
                    nc.gpsimd.dma_start(x8_t[:], d_major(x8))
                wv_t = xpool.tile([P, ND, D], BF16, tag="wv", name="wv")
                nc.scalar.dma_start(xT_t[:, :, 0:512], d_major(xT[:, 0:512]))
                nc.sync.dma_start(
                    xT_t[:, :, 512:1024], d_major(xT[:, 512:1024])
                )
                nc.gpsimd.dma_start(wv_t[:], d_major(wvT))
                nc.gpsimd.dma_start(
                    xT_t[:, :, 1024:2048], d_major(xT[:, 1024:2048])
                )

                # PE warm-up (covers engine/DMA spin-up + clock ramp)
                wps = psA.tile([P, 512], F32, tag="wps", name="wps", bufs=1)
                for w in range(48):
                    _mm(nc, wps[:], warm[:, 0:P], warm[:], start=True, stop=True)

                # --- z projection over own keys: zt[o, :, j_local] ---
                #     z = (M_SCALE * M) x^T, contraction over d
                for jc in range(JH // 512):
                    for o in range(NO):
                        pz = psA.tile([P, 512], F32, tag="pp", name=f"pz{jc}_{o}")
                        if Z_FP8:
                            for dp in range(ND // 2):
                                nc.tensor.matmul(
                                    pz[:],
                                    m8_t[:, dp, :, o * P:(o + 1) * P],
                                    x8_t[:, dp, :, jc * 512:(jc + 1) * 512],
                                    start=(dp == 0), stop=(dp == ND // 2 - 1),
                                    perf_mode=DR,
                                )
                        else:
                            for d in range(ND):
                                _mm(nc, pz[:],
                                    m8_t[:, d, o * P:(o + 1) * P],
                                    x8_t[:, d, jc * 512:(jc + 1) * 512],
                                    start=(d == 0), stop=(d == ND - 1))
                        if o % 2 == 0:
                            nc.vector.tensor_copy(
                                zt[:, o, jc * 512:(jc + 1) * 512], pz[:])
                        else:
                            nc.scalar.copy(
                                zt[:, o, jc * 512:(jc + 1) * 512], pz[:])

                # --- V projection over own keys: v[j 128, t, o] ---
                for t in range(NJT):
                    for ob in range(2):
                        pv = psA.tile([P, 512], F32, tag="pp", name=f"pv{t}_{ob}")
                        for d in range(ND):
                            _mm(nc, pv[:],
                                xT_t[:, d, t * P:(t + 1) * P],
                                wv_t[:, d, ob * 512:(ob + 1) * 512],
                                start=(d == 0), stop=(d == ND - 1))
                        if ob % 2 == 0:
                            nc.vector.tensor_copy(
                                v_t[:, t, ob * 512:(ob + 1) * 512], pv[:])
                        else:
                            nc.scalar.copy(
                                v_t[:, t, ob * 512:(ob + 1) * 512], pv[:])

            # ---------------- Phase B: attention ----------------
            # One flattened stream of (block, tile) steps; the AV+den
            # matmuls for step k are issued AFTER the scores of step k+1
            # (even across block boundaries), so every exp/mask chain and
            # every PSUM-bank WAR wait has a full scores window of cover.
            with (
                tc.tile_pool(name="ex", bufs=4) as expool,
                tc.tile_pool(name="psB", bufs=1, space="PSUM") as psB,
            ):
                steps = []
                # largest block first => the final block's exposed output
                # DMA is minimal; in-block order [0, diag, 1, ...]
                for ib in reversed(range(N_IB)):
                    njt = ib + 1
                    if njt >= 3:
                        proc = [0, njt - 1] + list(range(1, njt - 1))
                    else:
                        proc = list(range(njt))
                    for idx, t in enumerate(proc):
                        steps.append((ib, t, idx == 0, idx == njt - 1))

                state = {}   # ib -> (cps, dps)

                def issue_av(ib, t, first, last, et):
                    cps, dps = state[ib]
                    for it in range(2):
                        lhs = et[:, it * P:(it + 1) * P]
                        for ob in range(2):
                            _mm(nc, cps[it][ob][:], lhs,
                                v_t[:, t, ob * 512:(ob + 1) * 512],
                                start=first, stop=last)
                    # transposed den: ones^T @ et -> [2, 256], one matmul
                    _mm(nc, dps[:], ones_t[:], et[:], start=first, stop=last)
                    if not last:
                        return
                    # block done: drain ctx/den partials
                    nc.vector.tensor_copy(
                        den_all[:, ib * IB:(ib + 1) * IB], dps[:])
                    for it in range(2):
                        p_tile = it * 8 + ib          # permuted row-tile
                        row0 = p_tile * P
                        ot = expool.tile([P, D], BF16, tag="ot",
                                         name=f"ot{ib}_{it}")
                        for ob in range(2):
                            cols = slice(ob * 512, (ob + 1) * 512)
                            if it == 0:
                                nc.vector.tensor_copy(ot[:, cols],
                                                      cps[it][ob][:])
                            else:
                                nc.scalar.copy(ot[:, cols], cps[it][ob][:])
                            eng = nc.sync if (it + ob) % 2 == 0 else nc.gpsimd
                            eng.dma_start(
                                ctx_out[row0:row0 + P, cols], ot[:, cols])

                pending = None   # (ib, t, first, last, et)
                for ib, t, first, last in steps:
                    if first:
                        state[ib] = (
                            [
                                [
                                    psB.tile([P, 512], F32, tag=f"c{it}{ob}",
                                             name=f"c{ib}_{it}{ob}")
                                    for ob in range(2)
                                ]
                                for it in range(2)
                            ],
                            psB.tile([2, IB], F32, tag="dd",
                                     name=f"d{ib}", bufs=2),
                        )
                    ps = psB.tile([P, IB], F32, tag="ps",
                                  name=f"ps{ib}_{t}", bufs=2)
                    for o in range(NO):
                        qv = xT_t[:, o, :].rearrange(
                            "p (h u c) -> p h u c", h=2, c=P
                        )
                        _mm(nc, ps[:],
                            zt[:, o, t * P:(t + 1) * P],
                            qv[:, :, ib, :],
                            start=(o == 0), stop=(o == NO - 1))
                    et = expool.tile([P, IB], BF16, tag="et",
                                     name=f"et{ib}_{t}")
                    nc.scalar.activation(
                        et[:], ps[:],
                        mybir.ActivationFunctionType.Exp, scale=scale,
                    )
                    if t == ib:   # diagonal tile
                        etm = expool.tile([P, IB], BF16, tag="md",
                                          name=f"md{ib}")
                        nc.vector.tensor_mul(etm[:], et[:], mask_t[:])
                        et = etm
                    if pending is not None:
                        issue_av(*pending)
                    pending = (ib, t, first, last, et)
                issue_av(*pending)
                nc.scalar.dma_start(den_out[:], den_all[:])

    nc.compile()
    return nc


_NC_CACHE = None


def _get_nc():
    global _NC_CACHE
    if _NC_CACHE is None:
        _NC_CACHE = build_program()
    return _NC_CACHE


def _perm_rows(r):
    """Permuted row order: own j-tiles first, then the partner's."""
    own = [2 * t + r for t in range(NJT)]
    other = [2 * t + (1 - r) for t in range(NJT)]
    tiles = own + other
    return np.concatenate(
        [np.arange(g * P, (g + 1) * P) for g in tiles]
    )


def make_core_inputs(x, Wq, Wk, Wv):
    """Host-side shard prep. Returns list of 8 in_maps."""
    x = np.asarray(x, dtype=np.float32)
    # fold Wq into Wk:  scores = x (Wq^T Wk) x^T;  upload M^T d-major
    M = (np.asarray(Wq, np.float64).T @ np.asarray(Wk, np.float64))
    zdt = NPFP8 if Z_FP8 else NPBF16
    m8 = np.ascontiguousarray(M.T * M_SCALE).astype(np.float32).astype(zdt)
    wvT = np.ascontiguousarray(np.asarray(Wv, np.float32).T).astype(NPBF16)
    ones = np.ones((P, 2), NPBF16)

    # diagonal-tile masks [jj, ii] over i = [self-tile | partner-tile]:
    #   r=0: [tril | keep-all]   r=1: [tril | drop-all]
    jj = np.arange(P, dtype=np.float32)[:, None]
    ii = np.arange(P, dtype=np.float32)[None, :]
    tril = (jj <= ii).astype(NPBF16)
    masks = [
        np.concatenate([tril, np.ones((P, P), NPBF16)], axis=1),
        np.concatenate([tril, np.zeros((P, P), NPBF16)], axis=1),
    ]

    in_maps = []
    for c in range(N_CORES):
        b, r = divmod(c, 2)
        xp = x[b][_perm_rows(r), :]        # [S, D] fp32, permuted rows
        xpT = xp.T                          # [D, S]
        xT = np.ascontiguousarray(xpT).astype(NPBF16)
        x8 = np.ascontiguousarray(xpT[:, 0:JH]).astype(zdt)
        in_maps.append({
            "xT": xT, "x8": x8, "m8": m8, "wvT": wvT,
            "mask": masks[r], "ones": ones,
        })
    return in_maps


def assemble_output(results):
    """Combine per-core partial (ctx, den) into the full [B, S, D] output."""
    out = np.empty((B, S, D), np.float32)
    for b in range(B):
        num = np.zeros((S, D), np.float32)
        den = np.zeros((S, 1), np.float32)
        for r in range(2):
            res = results[2 * b + r]
            ctx_p = np.asarray(res["ctx"]).astype(np.float32)   # [S, D] perm
            den_p = np.asarray(res["den"])                       # [2, 16*IB]
            inv = _perm_rows(r)                # permuted pos -> global row
            num[inv] += ctx_p
            # den col j of block ib -> permuted tile (j//128)*8 + ib
            dv = np.empty((S,), np.float32)
            for ib in range(N_IB):
                blk = den_p[0, ib * IB:(ib + 1) * IB]
                for h in range(2):
                    p_tile = h * 8 + ib
                    dv[p_tile * P:(p_tile + 1) * P] = blk[h * P:(h + 1) * P]
            den[inv, 0] += dv
        out[b] = num / den
    return out


def kernel(x, Wq, Wk, Wv):
    nc = _get_nc()
    in_maps = make_core_inputs(x, Wq, Wk, Wv)
    res = run_bass_kernel_spmd(nc, in_maps, list(range(N_CORES)))
    return assemble_output(res.results)


# revision 27
# speedup vs baseline: 1.0611x; 1.0611x over previous
"""
Causal self-attention (single head) on 8 trn2 NeuronCores.

Problem: x[4, 2048, 1024], Wq/Wk/Wv[1024, 1024] (torch Linear layout
[d_out, d_in]).
    q/k/v = x @ W.T ; out = softmax(mask(q k^T) / 32) @ v

Key algebraic restructuring — fold Wq into Wk on the HOST:
    scores = (x Wq^T)(x Wk^T)^T = x (Wq^T Wk) x^T = x M x^T
M = Wq^T Wk is computed host-side in fp64 (1 GFLOP, free).  On-chip,
ONE projection  z = M x^T  over the core's own keys replaces BOTH the
q-projection (which was duplicated across the pair) and the
k-projection; the query side of the scores matmul reads the resident
xT tile directly.  This cuts the matmul work per core by ~1/3.

Sharding — flash-style key split (no collectives, uniform SPMD
program; all role differences live in the INPUTS):
  core c -> batch b = c // 2, role r = c % 2.
  Keys/values split by alternating 128-row j-tiles: core r owns global
  j-tiles {2t + r} and computes z/V only for its own 1024 key rows.
  Each core produces partial ctx = sum_j exp(s_j) v_j and partial
  den = sum_j exp(s_j) over ITS keys for ALL queries; the host
  combines:  out = (ctxE + ctxO) / (denE + denO).
  exp needs no running-max (logits/32 are O(2)).

  Host-side column permutation: x columns (sequence) are reordered
  OWN-tiles-first [own 8 x 128 | other 8 x 128].  z/V projections read
  xT[:, 0:1024] on every core (uniform program); query block ib is
  permuted row-tiles {ib, ib+8} on both roles, selected with one
  strided access pattern.  ctx/den come out in permuted row order; the
  host un-permutes.  The diagonal-tile causal mask is constant per
  role:  r=0: [tril | keep-all]   r=1: [tril | drop-all]  (bf16 0/1).

Precision: the z projection runs in fp8 e4m3 with the DoubleRow perf
mode (2 contraction tiles per instruction, 2x bf16 throughput; M is
pre-scaled by 32 to clear the e4m3 subnormal cliff, the exp scale
absorbs it).  Everything else is bf16 (~0.43 ns/row at N=512,
LDWEIGHTS hidden); PSUM accumulates fp32.  Simulated end-to-end
rel err: 1.56e-2 (Z_FP8) / 4.1e-3 (bf16 z) vs the 2e-2 gate; HW has
matched the numpy sim exactly on every previous config, and test.py
re-verifies on HW.  fp8 on scores or on the v path measured over the
gate — not used.

Schedule notes (from perfetto traces):
 - Engines/DMA only start ~8-10 us into the NEFF; the warm-up tile is
   memset on-chip and ~72 warm matmuls lift the PE clock gate through
   that window (shorter warm-ups let the clock drop and the whole
   kernel settles ~20% slow).
 - Measured per-queue DMA rates are wildly uneven: gpsimd (software
   DGE) ~200 GB/s, scalar ~80, sync ~44.  Critical loads (m8 + own-x8)
   ride gpsimd first; the rest are ordered by need date.
 - Attention is one flattened stream of (block, tile) steps,
   software-pipelined: AV+den of step k are issued AFTER the scores of
   step k+1 (across block boundaries), so every exp/mask chain and
   PSUM-bank WAR wait has a full scores window (~0.9 us) of cover
   (cross-engine semaphore observation latency is ~0.85 us).
 - i-blocks run largest-first so the final block's output DMA is tiny;
   in-block tile order is [0, diag, 1, ...]; ctx copies split
   vector/scalar; den partials accumulate in one resident SBUF tile,
   DMA'd once at the end.
"""

import sys

for _p in ("/opt/trn_rl_repo", "/root/.axon_site/_ro/trn_rl_repo"):
    if _p not in sys.path:
        sys.path.append(_p)

import numpy as np
import ml_dtypes

import concourse.bass as bass
import concourse.mybir as mybir
import concourse.tile as tile
from concourse import bacc
from concourse.bass_utils import run_bass_kernel_spmd

F32 = mybir.dt.float32
BF16 = mybir.dt.bfloat16
FP8 = mybir.dt.float8e4
NPBF16 = ml_dtypes.bfloat16
NPFP8 = ml_dtypes.float8_e4m3
DR = mybir.MatmulPerfMode.DoubleRow

Z_FP8 = True         # z = Mx^T in fp8-DoubleRow (1.56e-2) vs bf16 (4.1e-3)

B, S, D = 4, 2048, 1024
P = 128
ND = D // P          # 8 d-tiles (projection contraction)
NO = D // P          # 8 o-tiles
IB = 256             # query block rows
N_IB = S // IB       # 8 query blocks
JH = S // 2          # 1024 own key rows per core
NJT = JH // P        # 8 own j-tiles
N_CORES = 8
M_SCALE = 32.0 if Z_FP8 else 1.0


def _mm(nc, out, lhsT, rhs, start, stop):
    nc.tensor.matmul(out, lhsT, rhs, start=start, stop=stop)


def build_program():
    nc = bacc.Bacc(
        "TRN2",
        target_bir_lowering=False,
        debug=False,
        enable_asserts=False,
        num_devices=N_CORES,
    )
    # xT: full permuted sequence bf16 (scores query side + V projection);
    # x8/m8: z-projection operands (own key half only)
    xT = nc.dram_tensor("xT", [D, S], BF16, kind="ExternalInput").ap()
    zdt = FP8 if Z_FP8 else BF16
    x8 = nc.dram_tensor("x8", [D, JH], zdt, kind="ExternalInput").ap()
    m8 = nc.dram_tensor("m8", [D, D], zdt, kind="ExternalInput").ap()
    wvT = nc.dram_tensor("wvT", [D, D], BF16, kind="ExternalInput").ap()
    mask_in = nc.dram_tensor("mask", [P, IB], BF16, kind="ExternalInput").ap()
    ones_in = nc.dram_tensor("ones", [P, 2], BF16, kind="ExternalInput").ap()
    ctx_out = nc.dram_tensor("ctx", [S, D], BF16, kind="ExternalOutput").ap()
    den_out = nc.dram_tensor("den", [P, 32], F32, kind="ExternalOutput").ap()

    scale = 1.0 / (32.0 * M_SCALE)  # 1/sqrt(d_v), absorbing the M pre-scale

    def d_major(ap2d):
        # [ND*P, C] DRAM view -> [P, ND, C] (partition-major 3D AP)
        return ap2d.rearrange("(nd p) c -> p nd c", p=P)

    with tile.TileContext(nc) as tc:
        with tc.tile_pool(name="res", bufs=1) as rpool:
            # On-chip warm-up source: no DMA dependency, so the PE clock
            # gate lifts during the DMA spin-up dead window.
            warm = rpool.tile([P, 512], BF16, tag="warm", name="warm")
            nc.vector.memset(warm[:], 0.0)
            mask_t = rpool.tile([P, IB], BF16, tag="mask")
            nc.sync.dma_start(mask_t[:], mask_in[:])
            ones_t = rpool.tile([P, 2], BF16, tag="ones")
            nc.sync.dma_start(ones_t[:], ones_in[:])

            zt = rpool.tile([P, NO, JH], BF16, tag="zt", name="zt")
            v_t = rpool.tile([P, NJT, D], BF16, tag="v", name="v")
            den_all = rpool.tile([P, 32], F32, tag="den", name="den_all")
            xT_t = rpool.tile([P, ND, S], BF16, tag="xT", name="xT")

            # ---------------- Phase A: projections ----------------
            with (
                tc.tile_pool(name="xp", bufs=1) as xpool,
                tc.tile_pool(name="psA", bufs=4, space="PSUM") as psA,
            ):
                # Queue plan by need date (gpsimd is the fast queue).
                if Z_FP8:
                    x8_t = xpool.tile([P, ND // 2, 2, JH], FP8, tag="x8",
                                      name="x8")
                    x8_v = d_major(x8).rearrange(
                        "p (d2 two) c -> p d2 two c", two=2)
                    m8_t = xpool.tile([P, ND // 2, 2, D], FP8, tag="m8",
                                      name="m8")
                    m8_v = d_major(m8).rearrange(
                        "p (d2 two) c -> p d2 two c", two=2)
                    nc.gpsimd.dma_start(m8_t[:, 0:2], m8_v[:, 0:2])
                    nc.scalar.dma_start(m8_t[:, 2:3], m8_v[:, 2:3])
                    nc.sync.dma_start(m8_t[:, 3:4], m8_v[:, 3:4])
                    nc.gpsimd.dma_start(x8_t[:, :, :, 0:512],
                                        x8_v[:, :, :, 0:512])
                    nc.gpsimd.dma_start(x8_t[:, :, :, 512:JH],
                                        x8_v[:, :, :, 512:JH])
                else:
                    x8_t = xpool.tile([P, ND, JH], BF16, tag="x8", name="x8")
                    m8_t = xpool.tile([P, ND, D], BF16, tag="m8", name="m8")
                    nc.gpsimd.dma_start(m8_t[:], d_major(m8))
                    nc.gpsimd.dma_start(x8_t[:], d_major(x8))
                wv_t = xpool.tile([P, ND, D], BF16, tag="wv", name="wv")
                nc.scalar.dma_start(xT_t[:, :, 0:512], d_major(xT[:, 0:512]))
                nc.sync.dma_start(
                    xT_t[:, :, 512:1024], d_major(xT[:, 512:1024])
                )
                nc.gpsimd.dma_start(wv_t[:], d_major(wvT))
                nc.gpsimd.dma_start(
                    xT_t[:, :, 1024:2048], d_major(xT[:, 1024:2048])
                )

                # PE warm-up (covers engine/DMA spin-up + clock ramp)
                wps = psA.tile([P, 512], F32, tag="wps", name="wps", bufs=1)
                for w in range(48):
                    _mm(nc, wps[:], warm[:, 0:P], warm[:], start=True, stop=True)

                # --- z projection over own keys: zt[o, :, j_local] ---
                #     z = (M_SCALE * M) x^T, contraction over d
                for jc in range(JH // 512):
                    for o in range(NO):
                        pz = psA.tile([P, 512], F32, tag="pp", name=f"pz{jc}_{o}")
                        if Z_FP8:
                            for dp in range(ND // 2):
                                nc.tensor.matmul(
                                    pz[:],
                                    m8_t[:, dp, :, o * P:(o + 1) * P],
                                    x8_t[:, dp, :, jc * 512:(jc + 1) * 512],
                                    start=(dp == 0), stop=(dp == ND // 2 - 1),
                                    perf_mode=DR,
                                )
                        else:
                            for d in range(ND):
                                _mm(nc, pz[:],
                                    m8_t[:, d, o * P:(o + 1) * P],
                                    x8_t[:, d, jc * 512:(jc + 1) * 512],
                                    start=(d == 0), stop=(d == ND - 1))
                        if o % 2 == 0:
                            nc.vector.tensor_copy(
                                zt[:, o, jc * 512:(jc + 1) * 512], pz[:])
                        else:
                            nc.scalar.copy(
                                zt[:, o, jc * 512:(jc + 1) * 512], pz[:])

                # --- V projection over own keys: v[j 128, t, o] ---
                for t in range(NJT):
                    for ob in range(2):
                        pv = psA.tile([P, 512], F32, tag="pp", name=f"pv{t}_{ob}")
                        for d in range(ND):
                            _mm(nc, pv[:],
                                xT_t[:, d, t * P:(t + 1) * P],
                                wv_t[:, d, ob * 512:(ob + 1) * 512],
                                start=(d == 0), stop=(d == ND - 1))
                        if ob % 2 == 0:
                            nc.vector.tensor_copy(
                                v_t[:, t, ob * 512:(ob + 1) * 512], pv[:])
                        else:
                            nc.scalar.copy(
                                v_t[:, t, ob * 512:(ob + 1) * 512], pv[:])

            # ---------------- Phase B: attention ----------------
            # One flattened stream of (block, tile) steps; the AV+den
            # matmuls for step k are issued AFTER the scores of step k+1
            # (even across block boundaries), so every exp/mask chain and
            # every PSUM-bank WAR wait has a full scores window of cover.
            with (
                tc.tile_pool(name="ex", bufs=4) as expool,
                tc.tile_pool(name="psB", bufs=1, space="PSUM") as psB,
            ):
                steps = []
                # largest block first => the final block's exposed output
                # DMA is minimal; in-block order [0, diag, 1, ...]
                for ib in reversed(range(N_IB)):
                    njt = ib + 1
                    if njt >= 3:
                        proc = [0, njt - 1] + list(range(1, njt - 1))
                    else:
                        proc = list(range(njt))
                    for idx, t in enumerate(proc):
                        steps.append((ib, t, idx == 0, idx == njt - 1))

                state = {}   # ib -> (cps, dps)

                def issue_av(ib, t, first, last, et):
                    cps, dps = state[ib]
                    for it in range(2):
                        lhs = et[:, it * P:(it + 1) * P]
                        for ob in range(2):
                            _mm(nc, cps[it][ob][:], lhs,
                                v_t[:, t, ob * 512:(ob + 1) * 512],
                                start=first, stop=last)
                        _mm(nc, dps[it][:], lhs, ones_t[:],
                            start=first, stop=last)
                    if not last:
                        return
                    # block done: drain ctx/den partials
                    for it in range(2):
                        p_tile = it * 8 + ib          # permuted row-tile
                        row0 = p_tile * P
                        col = 2 * (2 * ib + it)
                        nc.vector.tensor_copy(
                            den_all[:, col:col + 2], dps[it][:])
                        ot = expool.tile([P, D], BF16, tag="ot",
                                         name=f"ot{ib}_{it}")
                        for ob in range(2):
                            cols = slice(ob * 512, (ob + 1) * 512)
                            if it == 0:
                                nc.vector.tensor_copy(ot[:, cols],
                                                      cps[it][ob][:])
                            else:
                                nc.scalar.copy(ot[:, cols], cps[it][ob][:])
                            eng = nc.sync if (it + ob) % 2 == 0 else nc.gpsimd
                            eng.dma_start(
                                ctx_out[row0:row0 + P, cols], ot[:, cols])

                pending = None   # (ib, t, first, last, et)
                for ib, t, first, last in steps:
                    if first:
                        state[ib] = (
                            [
                                [
                                    psB.tile([P, 512], F32, tag=f"c{it}{ob}",
                                             name=f"c{ib}_{it}{ob}")
                                    for ob in range(2)
                                ]
                                for it in range(2)
                            ],
                            [
                                psB.tile([P, 2], F32, tag=f"d{it}",
                                         name=f"d{ib}_{it}")
                                for it in range(2)
                            ],
                        )
                    ps = psB.tile([P, IB], F32, tag="ps",
                                  name=f"ps{ib}_{t}", bufs=2)
                    for o in range(NO):
                        qv = xT_t[:, o, :].rearrange(
                            "p (h u c) -> p h u c", h=2, c=P
                        )
                        _mm(nc, ps[:],
                            zt[:, o, t * P:(t + 1) * P],
                            qv[:, :, ib, :],
                            start=(o == 0), stop=(o == NO - 1))
                    et = expool.tile([P, IB], BF16, tag="et",
                                     name=f"et{ib}_{t}")
                    nc.scalar.activation(
                        et[:], ps[:],
                        mybir.ActivationFunctionType.Exp, scale=scale,
                    )
                    if t == ib:   # diagonal tile
                        etm = expool.tile([P, IB], BF16, tag="md",
                                          name=f"md{ib}")
                        nc.vector.tensor_mul(etm[:], et[:], mask_t[:])
                        et = etm
                    if pending is not None:
                        issue_av(*pending)
                    pending = (ib, t, first, last, et)
                issue_av(*pending)
                nc.scalar.dma_start(den_out[:], den_all[:])

    nc.compile()
    return nc


_NC_CACHE = None


def _get_nc():
    global _NC_CACHE
    if _NC_CACHE is None:
        _NC_CACHE = build_program()
    return _NC_CACHE


def _perm_rows(r):
    """Permuted row order: own j-tiles first, then the partner's."""
    own = [2 * t + r for t in range(NJT)]
    other = [2 * t + (1 - r) for t in range(NJT)]
    tiles = own + other
    return np.concatenate(
        [np.arange(g * P, (g + 1) * P) for g in tiles]
    )


def make_core_inputs(x, Wq, Wk, Wv):
    """Host-side shard prep. Returns list of 8 in_maps."""
    x = np.asarray(x, dtype=np.float32)
    # fold Wq into Wk:  scores = x (Wq^T Wk) x^T;  upload M^T d-major
    M = (np.asarray(Wq, np.float64).T @ np.asarray(Wk, np.float64))
    zdt = NPFP8 if Z_FP8 else NPBF16
    m8 = np.ascontiguousarray(M.T * M_SCALE).astype(np.float32).astype(zdt)
    wvT = np.ascontiguousarray(np.asarray(Wv, np.float32).T).astype(NPBF16)
    ones = np.ones((P, 2), NPBF16)

    # diagonal-tile masks [jj, ii] over i = [self-tile | partner-tile]:
    #   r=0: [tril | keep-all]   r=1: [tril | drop-all]
    jj = np.arange(P, dtype=np.float32)[:, None]
    ii = np.arange(P, dtype=np.float32)[None, :]
    tril = (jj <= ii).astype(NPBF16)
    masks = [
        np.concatenate([tril, np.ones((P, P), NPBF16)], axis=1),
        np.concatenate([tril, np.zeros((P, P), NPBF16)], axis=1),
    ]

    in_maps = []
    for c in range(N_CORES):
        b, r = divmod(c, 2)
        xp = x[b][_perm_rows(r), :]        # [S, D] fp32, permuted rows
        xpT = xp.T                          # [D, S]
        xT = np.ascontiguousarray(xpT).astype(NPBF16)
        x8 = np.ascontiguousarray(xpT[:, 0:JH]).astype(zdt)
        in_maps.append({
            "xT": xT, "x8": x8, "m8": m8, "wvT": wvT,
            "mask": masks[r], "ones": ones,
        })
    return in_maps


def assemble_output(results):
    """Combine per-core partial (ctx, den) into the full [B, S, D] output."""
    out = np.empty((B, S, D), np.float32)
    for b in range(B):
        num = np.zeros((S, D), np.float32)
        den = np.zeros((S, 1), np.float32)
        for r in range(2):
            res = results[2 * b + r]
            ctx_p = np.asarray(res["ctx"]).astype(np.float32)   # [S, D] perm
            den_p = np.asarray(res["den"])                       # [P, 32]
            inv = _perm_rows(r)                # permuted pos -> global row
            num[inv] += ctx_p
            # den slot for permuted tile p: p = it*8 + ib, col = 2*(2*ib+it)
            dv = np.empty((S,), np.float32)
            for ib in range(N_IB):
                for it in range(2):
                    p_tile = it * 8 + ib
                    col = 2 * (2 * ib + it)
                    dv[p_tile * P:(p_tile + 1) * P] = den_p[:, col]
            den[inv, 0] += dv
        out[b] = num / den
    return out


def kernel(x, Wq, Wk, Wv):
    nc = _get_nc()
    in_maps = make_core_inputs(x, Wq, Wk, Wv)
    res = run_bass_kernel_spmd(nc, in_maps, list(range(N_CORES)))
    return assemble_output(res.results)


# revision 28
# speedup vs baseline: 1.0636x; 1.0023x over previous
"""
Causal self-attention (single head) on 8 trn2 NeuronCores.

Problem: x[4, 2048, 1024], Wq/Wk/Wv[1024, 1024] (torch Linear layout
[d_out, d_in]).
    q/k/v = x @ W.T ; out = softmax(mask(q k^T) / 32) @ v

Key algebraic restructuring — fold Wq into Wk on the HOST:
    scores = (x Wq^T)(x Wk^T)^T = x (Wq^T Wk) x^T = x M x^T
M = Wq^T Wk is computed host-side in fp64 (1 GFLOP, free).  On-chip,
ONE projection  z = M x^T  over the core's own keys replaces BOTH the
q-projection (which was duplicated across the pair) and the
k-projection; the query side of the scores matmul reads the resident
xT tile directly.  This cuts the matmul work per core by ~1/3.

Sharding — flash-style key split (no collectives, uniform SPMD
program; all role differences live in the INPUTS):
  core c -> batch b = c // 2, role r = c % 2.
  Keys/values split by alternating 128-row j-tiles: core r owns global
  j-tiles {2t + r} and computes z/V only for its own 1024 key rows.
  Each core produces partial ctx = sum_j exp(s_j) v_j and partial
  den = sum_j exp(s_j) over ITS keys for ALL queries; the host
  combines:  out = (ctxE + ctxO) / (denE + denO).
  exp needs no running-max (logits/32 are O(2)).

  Host-side column permutation: x columns (sequence) are reordered
  OWN-tiles-first [own 8 x 128 | other 8 x 128].  z/V projections read
  xT[:, 0:1024] on every core (uniform program); query block ib is
  permuted row-tiles {ib, ib+8} on both roles, selected with one
  strided access pattern.  ctx/den come out in permuted row order; the
  host un-permutes.  The diagonal-tile causal mask is constant per
  role:  r=0: [tril | keep-all]   r=1: [tril | drop-all]  (bf16 0/1).

Precision: the z projection runs in fp8 e4m3 with the DoubleRow perf
mode (2 contraction tiles per instruction, 2x bf16 throughput; M is
pre-scaled by 32 to clear the e4m3 subnormal cliff, the exp scale
absorbs it).  Everything else is bf16 (~0.43 ns/row at N=512,
LDWEIGHTS hidden); PSUM accumulates fp32.  Simulated end-to-end
rel err: 1.56e-2 (Z_FP8) / 4.1e-3 (bf16 z) vs the 2e-2 gate; HW has
matched the numpy sim exactly on every previous config, and test.py
re-verifies on HW.  fp8 on scores or on the v path measured over the
gate — not used.

Schedule notes (from perfetto traces):
 - Engines/DMA only start ~8-10 us into the NEFF; the warm-up tile is
   memset on-chip and ~72 warm matmuls lift the PE clock gate through
   that window (shorter warm-ups let the clock drop and the whole
   kernel settles ~20% slow).
 - Measured per-queue DMA rates are wildly uneven: gpsimd (software
   DGE) ~200 GB/s, scalar ~80, sync ~44.  Critical loads (m8 + own-x8)
   ride gpsimd first; the rest are ordered by need date.
 - Attention is one flattened stream of (block, tile) steps,
   software-pipelined: AV+den of step k are issued AFTER the scores of
   step k+1 (across block boundaries), so every exp/mask chain and
   PSUM-bank WAR wait has a full scores window (~0.9 us) of cover
   (cross-engine semaphore observation latency is ~0.85 us).
 - i-blocks run largest-first so the final block's output DMA is tiny;
   in-block tile order is [0, diag, 1, ...]; ctx copies split
   vector/scalar; den partials accumulate in one resident SBUF tile,
   DMA'd once at the end.
"""

import sys

for _p in ("/opt/trn_rl_repo", "/root/.axon_site/_ro/trn_rl_repo"):
    if _p not in sys.path:
        sys.path.append(_p)

import numpy as np
import ml_dtypes

import concourse.bass as bass
import concourse.mybir as mybir
import concourse.tile as tile
from concourse import bacc
from concourse.bass_utils import run_bass_kernel_spmd

F32 = mybir.dt.float32
BF16 = mybir.dt.bfloat16
FP8 = mybir.dt.float8e4
NPBF16 = ml_dtypes.bfloat16
NPFP8 = ml_dtypes.float8_e4m3
DR = mybir.MatmulPerfMode.DoubleRow

Z_FP8 = True         # z = Mx^T in fp8-DoubleRow (1.56e-2) vs bf16 (4.1e-3)

B, S, D = 4, 2048, 1024
P = 128
ND = D // P          # 8 d-tiles (projection contraction)
NO = D // P          # 8 o-tiles
IB = 256             # query block rows
N_IB = S // IB       # 8 query blocks
JH = S // 2          # 1024 own key rows per core
NJT = JH // P        # 8 own j-tiles
N_CORES = 8
M_SCALE = 32.0 if Z_FP8 else 1.0


def _mm(nc, out, lhsT, rhs, start, stop):
    nc.tensor.matmul(out, lhsT, rhs, start=start, stop=stop)


def build_program():
    nc = bacc.Bacc(
        "TRN2",
        target_bir_lowering=False,
        debug=False,
        enable_asserts=False,
        num_devices=N_CORES,
    )
    # xT: full permuted sequence bf16 (scores query side + V projection);
    # x8/m8: z-projection operands (own key half only)
    xT = nc.dram_tensor("xT", [D, S], BF16, kind="ExternalInput").ap()
    zdt = FP8 if Z_FP8 else BF16
    x8 = nc.dram_tensor("x8", [D, JH], zdt, kind="ExternalInput").ap()
    m8 = nc.dram_tensor("m8", [D, D], zdt, kind="ExternalInput").ap()
    wvT = nc.dram_tensor("wvT", [D, D], BF16, kind="ExternalInput").ap()
    mask_in = nc.dram_tensor("mask", [P, IB], BF16, kind="ExternalInput").ap()
    ones_in = nc.dram_tensor("ones", [P, 2], BF16, kind="ExternalInput").ap()
    ctx_out = nc.dram_tensor("ctx", [S, D], BF16, kind="ExternalOutput").ap()
    den_out = nc.dram_tensor("den", [P, 32], F32, kind="ExternalOutput").ap()

    scale = 1.0 / (32.0 * M_SCALE)  # 1/sqrt(d_v), absorbing the M pre-scale

    def d_major(ap2d):
        # [ND*P, C] DRAM view -> [P, ND, C] (partition-major 3D AP)
        return ap2d.rearrange("(nd p) c -> p nd c", p=P)

    with tile.TileContext(nc) as tc:
        with tc.tile_pool(name="res", bufs=1) as rpool:
            # On-chip warm-up source: no DMA dependency, so the PE clock
            # gate lifts during the DMA spin-up dead window.
            warm = rpool.tile([P, 512], BF16, tag="warm", name="warm")
            nc.vector.memset(warm[:], 0.0)
            mask_t = rpool.tile([P, IB], BF16, tag="mask")
            nc.sync.dma_start(mask_t[:], mask_in[:])
            ones_t = rpool.tile([P, 2], BF16, tag="ones")
            nc.sync.dma_start(ones_t[:], ones_in[:])

            zt = rpool.tile([P, NO, JH], BF16, tag="zt", name="zt")
            v_t = rpool.tile([P, NJT, D], BF16, tag="v", name="v")
            den_all = rpool.tile([P, 32], F32, tag="den", name="den_all")
            xT_t = rpool.tile([P, ND, S], BF16, tag="xT", name="xT")

            # ---------------- Phase A: projections ----------------
            with (
                tc.tile_pool(name="xp", bufs=1) as xpool,
                tc.tile_pool(name="psA", bufs=4, space="PSUM") as psA,
            ):
                # Queue plan by need date (gpsimd is the fast queue).
                if Z_FP8:
                    x8_t = xpool.tile([P, ND // 2, 2, JH], FP8, tag="x8",
                                      name="x8")
                    x8_v = d_major(x8).rearrange(
                        "p (d2 two) c -> p d2 two c", two=2)
                    m8_t = xpool.tile([P, ND // 2, 2, D], FP8, tag="m8",
                                      name="m8")
                    m8_v = d_major(m8).rearrange(
                        "p (d2 two) c -> p d2 two c", two=2)
                    nc.gpsimd.dma_start(m8_t[:, 0:2], m8_v[:, 0:2])
                    nc.scalar.dma_start(m8_t[:, 2:3], m8_v[:, 2:3])
                    nc.sync.dma_start(m8_t[:, 3:4], m8_v[:, 3:4])
                    nc.gpsimd.dma_start(x8_t[:, :, :, 0:512],
                                        x8_v[:, :, :, 0:512])
                    nc.gpsimd.dma_start(x8_t[:, :, :, 512:JH],
                                        x8_v[:, :, :, 512:JH])
                else:
                    x8_t = xpool.tile([P, ND, JH], BF16, tag="x8", name="x8")
                    m8_t = xpool.tile([P, ND, D], BF16, tag="m8", name="m8")
                    nc.gpsimd.dma_start(m8_t[:], d_major(m8))
                    nc.gpsimd.dma_start(x8_t[:], d_major(x8))
                wv_t = xpool.tile([P, ND, D], BF16, tag="wv", name="wv")
                nc.scalar.dma_start(xT_t[:, :, 0:512], d_major(xT[:, 0:512]))
                nc.sync.dma_start(
                    xT_t[:, :, 512:1024], d_major(xT[:, 512:1024])
                )
                nc.gpsimd.dma_start(wv_t[:], d_major(wvT))
                nc.gpsimd.dma_start(
                    xT_t[:, :, 1024:2048], d_major(xT[:, 1024:2048])
                )

                # PE warm-up (covers engine/DMA spin-up + clock ramp)
                wps = psA.tile([P, 512], F32, tag="wps", name="wps", bufs=1)
                for w in range(44):
                    _mm(nc, wps[:], warm[:, 0:P], warm[:], start=True, stop=True)

                # --- z projection over own keys: zt[o, :, j_local] ---
                #     z = (M_SCALE * M) x^T, contraction over d
                for jc in range(JH // 512):
                    for o in range(NO):
                        pz = psA.tile([P, 512], F32, tag="pp", name=f"pz{jc}_{o}")
                        if Z_FP8:
                            for dp in range(ND // 2):
                                nc.tensor.matmul(
                                    pz[:],
                                    m8_t[:, dp, :, o * P:(o + 1) * P],
                                    x8_t[:, dp, :, jc * 512:(jc + 1) * 512],
                                    start=(dp == 0), stop=(dp == ND // 2 - 1),
                                    perf_mode=DR,
                                )
                        else:
                            for d in range(ND):
                                _mm(nc, pz[:],
                                    m8_t[:, d, o * P:(o + 1) * P],
                                    x8_t[:, d, jc * 512:(jc + 1) * 512],
                                    start=(d == 0), stop=(d == ND - 1))
                        if o % 2 == 0:
                            nc.vector.tensor_copy(
                                zt[:, o, jc * 512:(jc + 1) * 512], pz[:])
                        else:
                            nc.scalar.copy(
                                zt[:, o, jc * 512:(jc + 1) * 512], pz[:])

                # --- V projection over own keys: v[j 128, t, o] ---
                for t in range(NJT):
                    for ob in range(2):
                        pv = psA.tile([P, 512], F32, tag="pp", name=f"pv{t}_{ob}")
                        for d in range(ND):
                            _mm(nc, pv[:],
                                xT_t[:, d, t * P:(t + 1) * P],
                                wv_t[:, d, ob * 512:(ob + 1) * 512],
                                start=(d == 0), stop=(d == ND - 1))
                        if ob % 2 == 0:
                            nc.vector.tensor_copy(
                                v_t[:, t, ob * 512:(ob + 1) * 512], pv[:])
                        else:
                            nc.scalar.copy(
                                v_t[:, t, ob * 512:(ob + 1) * 512], pv[:])

            # ---------------- Phase B: attention ----------------
            # One flattened stream of (block, tile) steps; the AV+den
            # matmuls for step k are issued AFTER the scores of step k+1
            # (even across block boundaries), so every exp/mask chain and
            # every PSUM-bank WAR wait has a full scores window of cover.
            with (
                tc.tile_pool(name="ex", bufs=4) as expool,
                tc.tile_pool(name="psB", bufs=1, space="PSUM") as psB,
            ):
                steps = []
                # largest block first => the final block's exposed output
                # DMA is minimal; in-block order [0, diag, 1, ...]
                for ib in reversed(range(N_IB)):
                    njt = ib + 1
                    if njt >= 3:
                        proc = [0, njt - 1] + list(range(1, njt - 1))
                    else:
                        proc = list(range(njt))
                    for idx, t in enumerate(proc):
                        steps.append((ib, t, idx == 0, idx == njt - 1))

                state = {}   # ib -> (cps, dps)

                def issue_av(ib, t, first, last, et):
                    cps, dps = state[ib]
                    for it in range(2):
                        lhs = et[:, it * P:(it + 1) * P]
                        for ob in range(2):
                            _mm(nc, cps[it][ob][:], lhs,
                                v_t[:, t, ob * 512:(ob + 1) * 512],
                                start=first, stop=last)
                        _mm(nc, dps[it][:], lhs, ones_t[:],
                            start=first, stop=last)
                    if not last:
                        return
                    # block done: drain ctx/den partials
                    for it in range(2):
                        p_tile = it * 8 + ib          # permuted row-tile
                        row0 = p_tile * P
                        col = 2 * (2 * ib + it)
                        nc.vector.tensor_copy(
                            den_all[:, col:col + 2], dps[it][:])
                        ot = expool.tile([P, D], BF16, tag="ot",
                                         name=f"ot{ib}_{it}")
                        for ob in range(2):
                            cols = slice(ob * 512, (ob + 1) * 512)
                            if it == 0:
                                nc.vector.tensor_copy(ot[:, cols],
                                                      cps[it][ob][:])
                            else:
                                nc.scalar.copy(ot[:, cols], cps[it][ob][:])
                            eng = nc.sync if (it + ob) % 2 == 0 else nc.gpsimd
                            eng.dma_start(
                                ctx_out[row0:row0 + P, cols], ot[:, cols])

                pending = None   # (ib, t, first, last, et)
                for ib, t, first, last in steps:
                    if first:
                        state[ib] = (
                            [
                                [
                                    psB.tile([P, 512], F32, tag=f"c{it}{ob}",
                                             name=f"c{ib}_{it}{ob}")
                                    for ob in range(2)
                                ]
                                for it in range(2)
                            ],
                            [
                                psB.tile([P, 2], F32, tag=f"d{it}",
                                         name=f"d{ib}_{it}")
                                for it in range(2)
                            ],
                        )
                    ps = psB.tile([P, IB], F32, tag="ps",
                                  name=f"ps{ib}_{t}", bufs=2)
                    for o in range(NO):
                        qv = xT_t[:, o, :].rearrange(
                            "p (h u c) -> p h u c", h=2, c=P
                        )
                        _mm(nc, ps[:],
                            zt[:, o, t * P:(t + 1) * P],
                            qv[:, :, ib, :],
                            start=(o == 0), stop=(o == NO - 1))
                    et = expool.tile([P, IB], BF16, tag="et",
                                     name=f"et{ib}_{t}")
                    nc.scalar.activation(
                        et[:], ps[:],
                        mybir.ActivationFunctionType.Exp, scale=scale,
                    )
                    if t == ib:   # diagonal tile
                        etm = expool.tile([P, IB], BF16, tag="md",
                                          name=f"md{ib}")
                        nc.vector.tensor_mul(etm[:], et[:], mask_t[:])
                        et = etm
                    if pending is not None:
                        issue_av(*pending)
                    pending = (ib, t, first, last, et)
                issue_av(*pending)
                nc.scalar.dma_start(den_out[:], den_all[:])

    nc.compile()
    return nc


_NC_CACHE = None


def _get_nc():
    global _NC_CACHE
    if _NC_CACHE is None:
        _NC_CACHE = build_program()
    return _NC_CACHE


def _perm_rows(r):
    """Permuted row order: own j-tiles first, then the partner's."""
    own = [2 * t + r for t in range(NJT)]
    other = [2 * t + (1 - r) for t in range(NJT)]
    tiles = own + other
    return np.concatenate(
        [np.arange(g * P, (g + 1) * P) for g in tiles]
    )


def make_core_inputs(x, Wq, Wk, Wv):
    """Host-side shard prep. Returns list of 8 in_maps."""
    x = np.asarray(x, dtype=np.float32)
    # fold Wq into Wk:  scores = x (Wq^T Wk) x^T;  upload M^T d-major
    M = (np.asarray(Wq, np.float64).T @ np.asarray(Wk, np.float64))
    zdt = NPFP8 if Z_FP8 else NPBF16
    m8 = np.ascontiguousarray(M.T * M_SCALE).astype(np.float32).astype(zdt)
    wvT = np.ascontiguousarray(np.asarray(Wv, np.float32).T).astype(NPBF16)
    ones = np.ones((P, 2), NPBF16)

    # diagonal-tile masks [jj, ii] over i = [self-tile | partner-tile]:
    #   r=0: [tril | keep-all]   r=1: [tril | drop-all]
    jj = np.arange(P, dtype=np.float32)[:, None]
    ii = np.arange(P, dtype=np.float32)[None, :]
    tril = (jj <= ii).astype(NPBF16)
    masks = [
        np.concatenate([tril, np.ones((P, P), NPBF16)], axis=1),
        np.concatenate([tril, np.zeros((P, P), NPBF16)], axis=1),
    ]

    in_maps = []
    for c in range(N_CORES):
        b, r = divmod(c, 2)
        xp = x[b][_perm_rows(r), :]        # [S, D] fp32, permuted rows
        xpT = xp.T                          # [D, S]
        xT = np.ascontiguousarray(xpT).astype(NPBF16)
        x8 = np.ascontiguousarray(xpT[:, 0:JH]).astype(zdt)
        in_maps.append({
            "xT": xT, "x8": x8, "m8": m8, "wvT": wvT,
            "mask": masks[r], "ones": ones,
        })
    return in_maps


def assemble_output(results):
    """Combine per-core partial (ctx, den) into the full [B, S, D] output."""
    out = np.empty((B, S, D), np.float32)
    for b in range(B):
        num = np.zeros((S, D), np.float32)
        den = np.zeros((S, 1), np.float32)
        for r in range(2):
            res = results[2 * b + r]
            ctx_p = np.asarray(res["ctx"]).astype(np.float32)   # [S, D] perm
            den_p = np.asarray(res["den"])                       # [P, 32]
            inv = _perm_rows(r)                # permuted pos -> global row
            num[inv] += ctx_p
            # den slot for permuted tile p: p = it*8 + ib, col = 2*(2*ib+it)
            dv = np.empty((S,), np.float32)
            for ib in range(N_IB):
                for it in range(2):
                    p_tile = it * 8 + ib
                    col = 2 * (2 * ib + it)
                    dv[p_tile * P:(p_tile + 1) * P] = den_p[:, col]
            den[inv, 0] += dv
        out[b] = num / den
    return out


def kernel(x, Wq, Wk, Wv):
    nc = _get_nc()
    in_maps = make_core_inputs(x, Wq, Wk, Wv)
    res = run_bass_kernel_spmd(nc, in_maps, list(range(N_CORES)))
    return assemble_output(res.results)


# revision 29
# speedup vs baseline: 1.0701x; 1.0061x over previous
"""
Causal self-attention (single head) on 8 trn2 NeuronCores.

Problem: x[4, 2048, 1024], Wq/Wk/Wv[1024, 1024] (torch Linear layout
[d_out, d_in]).
    q/k/v = x @ W.T ; out = softmax(mask(q k^T) / 32) @ v

Key algebraic restructuring — fold Wq into Wk on the HOST:
    scores = (x Wq^T)(x Wk^T)^T = x (Wq^T Wk) x^T = x M x^T
M = Wq^T Wk is computed host-side in fp64 (1 GFLOP, free).  On-chip,
ONE projection  z = M x^T  over the core's own keys replaces BOTH the
q-projection (which was duplicated across the pair) and the
k-projection; the query side of the scores matmul reads the resident
xT tile directly.  This cuts the matmul work per core by ~1/3.

Sharding — flash-style key split (no collectives, uniform SPMD
program; all role differences live in the INPUTS):
  core c -> batch b = c // 2, role r = c % 2.
  Keys/values split by alternating 128-row j-tiles: core r owns global
  j-tiles {2t + r} and computes z/V only for its own 1024 key rows.
  Each core produces partial ctx = sum_j exp(s_j) v_j and partial
  den = sum_j exp(s_j) over ITS keys for ALL queries; the host
  combines:  out = (ctxE + ctxO) / (denE + denO).
  exp needs no running-max (logits/32 are O(2)).

  Host-side column permutation: x columns (sequence) are reordered
  OWN-tiles-first [own 8 x 128 | other 8 x 128].  z/V projections read
  xT[:, 0:1024] on every core (uniform program); query block ib is
  permuted row-tiles {ib, ib+8} on both roles, selected with one
  strided access pattern.  ctx/den come out in permuted row order; the
  host un-permutes.  The diagonal-tile causal mask is constant per
  role:  r=0: [tril | keep-all]   r=1: [tril | drop-all]  (bf16 0/1).

Precision: the z projection runs in fp8 e4m3 with the DoubleRow perf
mode (2 contraction tiles per instruction, 2x bf16 throughput; M is
pre-scaled by 32 to clear the e4m3 subnormal cliff, the exp scale
absorbs it).  Everything else is bf16 (~0.43 ns/row at N=512,
LDWEIGHTS hidden); PSUM accumulates fp32.  Simulated end-to-end
rel err: 1.56e-2 (Z_FP8) / 4.1e-3 (bf16 z) vs the 2e-2 gate; HW has
matched the numpy sim exactly on every previous config, and test.py
re-verifies on HW.  fp8 on scores or on the v path measured over the
gate — not used.

Schedule notes (from perfetto traces):
 - Engines/DMA only start ~8-10 us into the NEFF; the warm-up tile is
   memset on-chip and ~72 warm matmuls lift the PE clock gate through
   that window (shorter warm-ups let the clock drop and the whole
   kernel settles ~20% slow).
 - Measured per-queue DMA rates are wildly uneven: gpsimd (software
   DGE) ~200 GB/s, scalar ~80, sync ~44.  Critical loads (m8 + own-x8)
   ride gpsimd first; the rest are ordered by need date.
 - Attention is one flattened stream of (block, tile) steps,
   software-pipelined: AV+den of step k are issued AFTER the scores of
   step k+1 (across block boundaries), so every exp/mask chain and
   PSUM-bank WAR wait has a full scores window (~0.9 us) of cover
   (cross-engine semaphore observation latency is ~0.85 us).
 - i-blocks run largest-first so the final block's output DMA is tiny;
   in-block tile order is [0, diag, 1, ...]; ctx copies split
   vector/scalar; den partials accumulate in one resident SBUF tile,
   DMA'd once at the end.
"""

import sys

for _p in ("/opt/trn_rl_repo", "/root/.axon_site/_ro/trn_rl_repo"):
    if _p not in sys.path:
        sys.path.append(_p)

import numpy as np
import ml_dtypes

import concourse.bass as bass
import concourse.mybir as mybir
import concourse.tile as tile
from concourse import bacc
from concourse.bass_utils import run_bass_kernel_spmd

F32 = mybir.dt.float32
BF16 = mybir.dt.bfloat16
FP8 = mybir.dt.float8e4
NPBF16 = ml_dtypes.bfloat16
NPFP8 = ml_dtypes.float8_e4m3
DR = mybir.MatmulPerfMode.DoubleRow

Z_FP8 = True         # z = Mx^T in fp8-DoubleRow (1.56e-2) vs bf16 (4.1e-3)

B, S, D = 4, 2048, 1024
P = 128
ND = D // P          # 8 d-tiles (projection contraction)
NO = D // P          # 8 o-tiles
IB = 256             # query block rows
N_IB = S // IB       # 8 query blocks
JH = S // 2          # 1024 own key rows per core
NJT = JH // P        # 8 own j-tiles
N_CORES = 8
M_SCALE = 32.0 if Z_FP8 else 1.0


def _mm(nc, out, lhsT, rhs, start, stop):
    nc.tensor.matmul(out, lhsT, rhs, start=start, stop=stop)


def build_program():
    nc = bacc.Bacc(
        "TRN2",
        target_bir_lowering=False,
        debug=False,
        enable_asserts=False,
        num_devices=N_CORES,
    )
    # xT: full permuted sequence bf16 (scores query side + V projection);
    # x8/m8: z-projection operands (own key half only)
    xT = nc.dram_tensor("xT", [D, S], BF16, kind="ExternalInput").ap()
    zdt = FP8 if Z_FP8 else BF16
    x8 = nc.dram_tensor("x8", [D, JH], zdt, kind="ExternalInput").ap()
    m8 = nc.dram_tensor("m8", [D, D], zdt, kind="ExternalInput").ap()
    wvT = nc.dram_tensor("wvT", [D, D], BF16, kind="ExternalInput").ap()
    mask_in = nc.dram_tensor("mask", [P, IB], BF16, kind="ExternalInput").ap()
    ones_in = nc.dram_tensor("ones", [P, 2], BF16, kind="ExternalInput").ap()
    ctx_out = nc.dram_tensor("ctx", [S, D], BF16, kind="ExternalOutput").ap()
    den_out = nc.dram_tensor("den", [P, 32], F32, kind="ExternalOutput").ap()

    scale = 1.0 / (32.0 * M_SCALE)  # 1/sqrt(d_v), absorbing the M pre-scale

    def d_major(ap2d):
        # [ND*P, C] DRAM view -> [P, ND, C] (partition-major 3D AP)
        return ap2d.rearrange("(nd p) c -> p nd c", p=P)

    with tile.TileContext(nc) as tc:
        with tc.tile_pool(name="res", bufs=1) as rpool:
            # On-chip warm-up source: no DMA dependency, so the PE clock
            # gate lifts during the DMA spin-up dead window.
            warm = rpool.tile([P, 512], BF16, tag="warm", name="warm")
            nc.vector.memset(warm[:], 0.0)
            mask_t = rpool.tile([P, IB], BF16, tag="mask")
            nc.sync.dma_start(mask_t[:], mask_in[:])
            ones_t = rpool.tile([P, 2], BF16, tag="ones")
            nc.sync.dma_start(ones_t[:], ones_in[:])

            zt = rpool.tile([P, NO, JH], BF16, tag="zt", name="zt")
            v_t = rpool.tile([P, NJT, D], BF16, tag="v", name="v")
            den_all = rpool.tile([P, 32], F32, tag="den", name="den_all")
            xT_t = rpool.tile([P, ND, S], BF16, tag="xT", name="xT")

            # ---------------- Phase A: projections ----------------
            with (
                tc.tile_pool(name="xp", bufs=1) as xpool,
                tc.tile_pool(name="psA", bufs=4, space="PSUM") as psA,
            ):
                # Queue plan by need date (gpsimd is the fast queue).
                if Z_FP8:
                    x8_t = xpool.tile([P, ND // 2, 2, JH], FP8, tag="x8",
                                      name="x8")
                    x8_v = d_major(x8).rearrange(
                        "p (d2 two) c -> p d2 two c", two=2)
                    m8_t = xpool.tile([P, ND // 2, 2, D], FP8, tag="m8",
                                      name="m8")
                    m8_v = d_major(m8).rearrange(
                        "p (d2 two) c -> p d2 two c", two=2)
                    nc.gpsimd.dma_start(m8_t[:, 0:2], m8_v[:, 0:2])
                    nc.scalar.dma_start(m8_t[:, 2:3], m8_v[:, 2:3])
                    nc.sync.dma_start(m8_t[:, 3:4], m8_v[:, 3:4])
                    nc.gpsimd.dma_start(x8_t[:, :, :, 0:512],
                                        x8_v[:, :, :, 0:512])
                    nc.gpsimd.dma_start(x8_t[:, :, :, 512:JH],
                                        x8_v[:, :, :, 512:JH])
                else:
                    x8_t = xpool.tile([P, ND, JH], BF16, tag="x8", name="x8")
                    m8_t = xpool.tile([P, ND, D], BF16, tag="m8", name="m8")
                    nc.gpsimd.dma_start(m8_t[:], d_major(m8))
                    nc.gpsimd.dma_start(x8_t[:], d_major(x8))
                wv_t = xpool.tile([P, ND, D], BF16, tag="wv", name="wv")
                nc.scalar.dma_start(xT_t[:, :, 0:512], d_major(xT[:, 0:512]))
                nc.sync.dma_start(
                    xT_t[:, :, 512:1024], d_major(xT[:, 512:1024])
                )
                nc.gpsimd.dma_start(wv_t[:], d_major(wvT))
                nc.gpsimd.dma_start(
                    xT_t[:, :, 1024:2048], d_major(xT[:, 1024:2048])
                )

                # PE warm-up (covers engine/DMA spin-up + clock ramp)
                wps = psA.tile([P, 512], F32, tag="wps", name="wps", bufs=1)
                for w in range(40):
                    _mm(nc, wps[:], warm[:, 0:P], warm[:], start=True, stop=True)

                # --- z projection over own keys: zt[o, :, j_local] ---
                #     z = (M_SCALE * M) x^T, contraction over d
                for jc in range(JH // 512):
                    for o in range(NO):
                        pz = psA.tile([P, 512], F32, tag="pp", name=f"pz{jc}_{o}")
                        if Z_FP8:
                            for dp in range(ND // 2):
                                nc.tensor.matmul(
                                    pz[:],
                                    m8_t[:, dp, :, o * P:(o + 1) * P],
                                    x8_t[:, dp, :, jc * 512:(jc + 1) * 512],
                                    start=(dp == 0), stop=(dp == ND // 2 - 1),
                                    perf_mode=DR,
                                )
                        else:
                            for d in range(ND):
                                _mm(nc, pz[:],
                                    m8_t[:, d, o * P:(o + 1) * P],
                                    x8_t[:, d, jc * 512:(jc + 1) * 512],
                                    start=(d == 0), stop=(d == ND - 1))
                        if o % 2 == 0:
                            nc.vector.tensor_copy(
                                zt[:, o, jc * 512:(jc + 1) * 512], pz[:])
                        else:
                            nc.scalar.copy(
                                zt[:, o, jc * 512:(jc + 1) * 512], pz[:])

                # --- V projection over own keys: v[j 128, t, o] ---
                for t in range(NJT):
                    for ob in range(2):
                        pv = psA.tile([P, 512], F32, tag="pp", name=f"pv{t}_{ob}")
                        for d in range(ND):
                            _mm(nc, pv[:],
                                xT_t[:, d, t * P:(t + 1) * P],
                                wv_t[:, d, ob * 512:(ob + 1) * 512],
                                start=(d == 0), stop=(d == ND - 1))
                        if ob % 2 == 0:
                            nc.vector.tensor_copy(
                                v_t[:, t, ob * 512:(ob + 1) * 512], pv[:])
                        else:
                            nc.scalar.copy(
                                v_t[:, t, ob * 512:(ob + 1) * 512], pv[:])

            # ---------------- Phase B: attention ----------------
            # One flattened stream of (block, tile) steps; the AV+den
            # matmuls for step k are issued AFTER the scores of step k+1
            # (even across block boundaries), so every exp/mask chain and
            # every PSUM-bank WAR wait has a full scores window of cover.
            with (
                tc.tile_pool(name="ex", bufs=4) as expool,
                tc.tile_pool(name="psB", bufs=1, space="PSUM") as psB,
            ):
                steps = []
                # largest block first => the final block's exposed output
                # DMA is minimal; in-block order [0, diag, 1, ...]
                for ib in reversed(range(N_IB)):
                    njt = ib + 1
                    if njt >= 3:
                        proc = [0, njt - 1] + list(range(1, njt - 1))
                    else:
                        proc = list(range(njt))
                    for idx, t in enumerate(proc):
                        steps.append((ib, t, idx == 0, idx == njt - 1))

                state = {}   # ib -> (cps, dps)

                def issue_av(ib, t, first, last, et):
                    cps, dps = state[ib]
                    for it in range(2):
                        lhs = et[:, it * P:(it + 1) * P]
                        for ob in range(2):
                            _mm(nc, cps[it][ob][:], lhs,
                                v_t[:, t, ob * 512:(ob + 1) * 512],
                                start=first, stop=last)
                        _mm(nc, dps[it][:], lhs, ones_t[:],
                            start=first, stop=last)
                    if not last:
                        return
                    # block done: drain ctx/den partials
                    for it in range(2):
                        p_tile = it * 8 + ib          # permuted row-tile
                        row0 = p_tile * P
                        col = 2 * (2 * ib + it)
                        nc.vector.tensor_copy(
                            den_all[:, col:col + 2], dps[it][:])
                        ot = expool.tile([P, D], BF16, tag="ot",
                                         name=f"ot{ib}_{it}")
                        for ob in range(2):
                            cols = slice(ob * 512, (ob + 1) * 512)
                            if it == 0:
                                nc.vector.tensor_copy(ot[:, cols],
                                                      cps[it][ob][:])
                            else:
                                nc.scalar.copy(ot[:, cols], cps[it][ob][:])
                            eng = nc.sync if (it + ob) % 2 == 0 else nc.gpsimd
                            eng.dma_start(
                                ctx_out[row0:row0 + P, cols], ot[:, cols])

                pending = None   # (ib, t, first, last, et)
                for ib, t, first, last in steps:
                    if first:
                        state[ib] = (
                            [
                                [
                                    psB.tile([P, 512], F32, tag=f"c{it}{ob}",
                                             name=f"c{ib}_{it}{ob}")
                                    for ob in range(2)
                                ]
                                for it in range(2)
                            ],
                            [
                                psB.tile([P, 2], F32, tag=f"d{it}",
                                         name=f"d{ib}_{it}")
                                for it in range(2)
                            ],
                        )
                    ps = psB.tile([P, IB], F32, tag="ps",
                                  name=f"ps{ib}_{t}", bufs=2)
                    for o in range(NO):
                        qv = xT_t[:, o, :].rearrange(
                            "p (h u c) -> p h u c", h=2, c=P
                        )
                        _mm(nc, ps[:],
                            zt[:, o, t * P:(t + 1) * P],
                            qv[:, :, ib, :],
                            start=(o == 0), stop=(o == NO - 1))
                    et = expool.tile([P, IB], BF16, tag="et",
                                     name=f"et{ib}_{t}")
                    nc.scalar.activation(
                        et[:], ps[:],
                        mybir.ActivationFunctionType.Exp, scale=scale,
                    )
                    if t == ib:   # diagonal tile
                        etm = expool.tile([P, IB], BF16, tag="md",
                                          name=f"md{ib}")
                        nc.vector.tensor_mul(etm[:], et[:], mask_t[:])
                        et = etm
                    if pending is not None:
                        issue_av(*pending)
                    pending = (ib, t, first, last, et)
                issue_av(*pending)
                nc.scalar.dma_start(den_out[:], den_all[:])

    nc.compile()
    return nc


_NC_CACHE = None


def _get_nc():
    global _NC_CACHE
    if _NC_CACHE is None:
        _NC_CACHE = build_program()
    return _NC_CACHE


def _perm_rows(r):
    """Permuted row order: own j-tiles first, then the partner's."""
    own = [2 * t + r for t in range(NJT)]
    other = [2 * t + (1 - r) for t in range(NJT)]
    tiles = own + other
    return np.concatenate(
        [np.arange(g * P, (g + 1) * P) for g in tiles]
    )


def make_core_inputs(x, Wq, Wk, Wv):
    """Host-side shard prep. Returns list of 8 in_maps."""
    x = np.asarray(x, dtype=np.float32)
    # fold Wq into Wk:  scores = x (Wq^T Wk) x^T;  upload M^T d-major
    M = (np.asarray(Wq, np.float64).T @ np.asarray(Wk, np.float64))
    zdt = NPFP8 if Z_FP8 else NPBF16
    m8 = np.ascontiguousarray(M.T * M_SCALE).astype(np.float32).astype(zdt)
    wvT = np.ascontiguousarray(np.asarray(Wv, np.float32).T).astype(NPBF16)
    ones = np.ones((P, 2), NPBF16)

    # diagonal-tile masks [jj, ii] over i = [self-tile | partner-tile]:
    #   r=0: [tril | keep-all]   r=1: [tril | drop-all]
    jj = np.arange(P, dtype=np.float32)[:, None]
    ii = np.arange(P, dtype=np.float32)[None, :]
    tril = (jj <= ii).astype(NPBF16)
    masks = [
        np.concatenate([tril, np.ones((P, P), NPBF16)], axis=1),
        np.concatenate([tril, np.zeros((P, P), NPBF16)], axis=1),
    ]

    in_maps = []
    for c in range(N_CORES):
        b, r = divmod(c, 2)
        xp = x[b][_perm_rows(r), :]        # [S, D] fp32, permuted rows
        xpT = xp.T                          # [D, S]
        xT = np.ascontiguousarray(xpT).astype(NPBF16)
        x8 = np.ascontiguousarray(xpT[:, 0:JH]).astype(zdt)
        in_maps.append({
            "xT": xT, "x8": x8, "m8": m8, "wvT": wvT,
            "mask": masks[r], "ones": ones,
        })
    return in_maps


def assemble_output(results):
    """Combine per-core partial (ctx, den) into the full [B, S, D] output."""
    out = np.empty((B, S, D), np.float32)
    for b in range(B):
        num = np.zeros((S, D), np.float32)
        den = np.zeros((S, 1), np.float32)
        for r in range(2):
            res = results[2 * b + r]
            ctx_p = np.asarray(res["ctx"]).astype(np.float32)   # [S, D] perm
            den_p = np.asarray(res["den"])                       # [P, 32]
            inv = _perm_rows(r)                # permuted pos -> global row
            num[inv] += ctx_p
            # den slot for permuted tile p: p = it*8 + ib, col = 2*(2*ib+it)
            dv = np.empty((S,), np.float32)
            for ib in range(N_IB):
                for it in range(2):
                    p_tile = it * 8 + ib
                    col = 2 * (2 * ib + it)
                    dv[p_tile * P:(p_tile + 1) * P] = den_p[:, col]
            den[inv, 0] += dv
        out[b] = num / den
    return out


def kernel(x, Wq, Wk, Wv):
    nc = _get_nc()
    in_maps = make_core_inputs(x, Wq, Wk, Wv)
    res = run_bass_kernel_spmd(nc, in_maps, list(range(N_CORES)))
    return assemble_output(res.results)
